# revision 1
# baseline (speedup 1.0000x reference)
"""LoRA embedding lookup on 8 Trainium2 NeuronCores.

out[b, s, :] = weight[ids[b, s], :] + SCALING * (lora_B[ids[b, s], :] @ lora_A)

Sharding: tokens are split across the 8 cores (batch row c -> core c).
Each core holds the full weight / lora_B tables in its HBM, gathers its
2048 rows with indirect DMA, runs the rank-16 delta matmul on the PE,
adds, and writes a disjoint slice of the output. No collectives needed.
"""

import numpy as np

try:
    import concourse.bass as bass
except ImportError:  # fresh grading dir without the default PYTHONPATH
    import sys

    sys.path.insert(0, "/opt/trn_rl_repo")
    import concourse.bass as bass

import concourse.mybir as mybir
import concourse.tile as tile
from concourse import bacc
from concourse.bass_utils import run_bass_kernel_spmd

VOCAB = 50257
DIM = 1024
RANK = 16
SCALING = 32.0 / 16.0  # alpha / rank
N_CORES = 8
TOK_PER_CORE = 2048
P = 128
N_TILES = TOK_PER_CORE // P  # 16
NSPLIT = 2  # PSUM bank limit: f32 matmul N <= 512

_cached_nc = None


def _build_nc():
    global _cached_nc
    if _cached_nc is not None:
        return _cached_nc

    nc = bacc.Bacc(None, target_bir_lowering=False)
    ids_d = nc.declare_dram_parameter("ids", [P, N_TILES], mybir.dt.int32, isOutput=False)
    w_d = nc.declare_dram_parameter("weight", [VOCAB, DIM], mybir.dt.float32, isOutput=False)
    a_d = nc.declare_dram_parameter("lora_a", [RANK, DIM], mybir.dt.float32, isOutput=False)
    b_d = nc.declare_dram_parameter("lora_b", [VOCAB, RANK], mybir.dt.float32, isOutput=False)
    out_d = nc.declare_dram_parameter("out", [TOK_PER_CORE, DIM], mybir.dt.float32, isOutput=True)

    f32 = mybir.dt.float32

    with tile.TileContext(nc) as tc:
        with (
            tc.tile_pool(name="const", bufs=1) as const_tp,
            tc.tile_pool(name="wp", bufs=4) as wp,
            tc.tile_pool(name="bp", bufs=4) as bp,
            tc.tile_pool(name="btp", bufs=4) as btp,
            tc.tile_pool(name="op", bufs=4) as op,
            tc.tile_pool(name="pst", bufs=2, space="PSUM") as pst,
            tc.tile_pool(name="psd", bufs=2, space="PSUM") as psd,
        ):
            from concourse.masks import make_identity

            identity = const_tp.tile([P, P], f32)
            make_identity(nc, identity[:])

            ids_sb = const_tp.tile([P, N_TILES], mybir.dt.int32)
            nc.sync.dma_start(out=ids_sb[:], in_=ids_d[:])

            a_sb = const_tp.tile([RANK, DIM], f32)
            nc.sync.dma_start(out=a_sb[:], in_=a_d[:])
            a_scaled = const_tp.tile([RANK, DIM], f32)
            nc.vector.tensor_scalar_mul(a_scaled[:], a_sb[:], SCALING)

            for j in range(N_TILES):
                # Gather 128 weight rows (one per partition) for this token tile.
                w_tile = wp.tile([P, DIM], f32)
                nc.gpsimd.indirect_dma_start(
                    out=w_tile[:],
                    out_offset=None,
                    in_=w_d[:],
                    in_offset=bass.IndirectOffsetOnAxis(ap=ids_sb[:, j : j + 1], axis=0),
                )
                # Gather the matching 128 lora_B rows.
                b_tile = bp.tile([P, RANK], f32)
                nc.gpsimd.indirect_dma_start(
                    out=b_tile[:],
                    out_offset=None,
                    in_=b_d[:],
                    in_offset=bass.IndirectOffsetOnAxis(ap=ids_sb[:, j : j + 1], axis=0),
                )
                # bT = b_tile.T : [RANK, P] so tokens land on PSUM partitions.
                bT_ps = pst.tile([RANK, P], f32)
                nc.tensor.transpose(out=bT_ps[:], in_=b_tile[:], identity=identity[:])
                bT = btp.tile([RANK, P], f32)
                nc.vector.tensor_copy(out=bT[:], in_=bT_ps[:])

                # delta = b_tile @ (SCALING * lora_A) : [P, DIM]
                d_ps = psd.tile([P, DIM], f32)
                for h in range(NSPLIT):
                    sl = slice(h * (DIM // NSPLIT), (h + 1) * (DIM // NSPLIT))
                    nc.tensor.matmul(
                        d_ps[:, sl],
                        bT[:],
                        a_scaled[:, sl],
                        start=True,
                        stop=True,
                    )

                out_tile = op.tile([P, DIM], f32)
                nc.vector.tensor_add(out=out_tile[:], in0=w_tile[:], in1=d_ps[:])
                nc.sync.dma_start(out=out_d[j * P : (j + 1) * P, :], in_=out_tile[:])

    nc.compile()
    _cached_nc = nc
    return nc


def run(inputs, **spmd_kwargs):
    """Run on 8 cores; returns (full_output, BassKernelResults)."""
    ids = np.ascontiguousarray(np.asarray(inputs["input_ids"]).astype(np.int32)).reshape(-1)
    weight = np.ascontiguousarray(np.asarray(inputs["weight"], dtype=np.float32))
    lora_a = np.ascontiguousarray(np.asarray(inputs["lora_A"], dtype=np.float32))
    lora_b = np.ascontiguousarray(np.asarray(inputs["lora_B"], dtype=np.float32))
    assert ids.shape == (N_CORES * TOK_PER_CORE,)
    assert weight.shape == (VOCAB, DIM)
    assert lora_a.shape == (RANK, DIM)
    assert lora_b.shape == (VOCAB, RANK)

    nc = _build_nc()
    in_maps = []
    for c in range(N_CORES):
        chunk = ids[c * TOK_PER_CORE : (c + 1) * TOK_PER_CORE]
        # ids_dev[p, j] = chunk[j * P + p] -> tile j gathers tokens j*P .. j*P+127
        ids_dev = np.ascontiguousarray(chunk.reshape(N_TILES, P).T)
        in_maps.append(
            {"ids": ids_dev, "weight": weight, "lora_a": lora_a, "lora_b": lora_b}
        )
    res = run_bass_kernel_spmd(nc, in_maps, list(range(N_CORES)), **spmd_kwargs)
    out = np.stack([res.results[c]["out"] for c in range(N_CORES)], axis=0)
    return out.astype(np.float32, copy=False), res


def kernel(**inputs):
    out, _ = run(inputs)
    return out


# revision 6
# speedup vs baseline: 1.4876x; 1.4876x over previous
"""LoRA embedding lookup on 8 Trainium2 NeuronCores.

out[b, s, :] = weight[ids[b, s], :] + SCALING * (lora_B[ids[b, s], :] @ lora_A)

Sharding: tokens are split across the 8 cores (batch row c -> core c).
Each core holds the full tables in its HBM, gathers its 2048 rows with
indirect DMA, runs the rank-16 delta matmul on the PE, adds, and writes
a disjoint slice of the output. No collectives needed.

Device-side layout tricks:
- weight and lora_B are fused host-side into one [VOCAB, 1040] table so a
  single indirect-DMA descriptor per token fetches both the embedding row
  and its LoRA-B row (halves SWDGE descriptor-generation work).
- The rank-16 delta matmul runs in bf16 on the PE (1 pass instead of
  fp32's 4); the dominant embedding term stays exact f32.
"""

import numpy as np

try:
    import concourse.bass as bass
except ImportError:  # fresh grading dir without the default PYTHONPATH
    import sys

    sys.path.insert(0, "/opt/trn_rl_repo")
    import concourse.bass as bass

import concourse.mybir as mybir
import concourse.tile as tile
from concourse import bacc
from concourse.bass_utils import run_bass_kernel_spmd

VOCAB = 50257
DIM = 1024
RANK = 16
ROW = DIM + RANK  # fused table row: [weight_row | lora_b_row]
SCALING = 32.0 / 16.0  # alpha / rank
N_CORES = 8
TOK_PER_CORE = 2048
P = 128
N_TILES = TOK_PER_CORE // P  # 16
GROUP = 1  # token tiles per indirect DMA (HW consumes one index per partition)
NSPLIT = 2  # PSUM bank limit: matmul N <= 512

_cached_nc = None


def _build_nc():
    global _cached_nc
    if _cached_nc is not None:
        return _cached_nc

    f32 = mybir.dt.float32
    bf16 = mybir.dt.bfloat16

    nc = bacc.Bacc(None, target_bir_lowering=False)
    ids_d = nc.declare_dram_parameter("ids", [P, N_TILES], mybir.dt.int32, isOutput=False)
    t_d = nc.declare_dram_parameter("table", [VOCAB, ROW], f32, isOutput=False)
    a_d = nc.declare_dram_parameter("lora_a", [RANK, DIM], f32, isOutput=False)
    out_d = nc.declare_dram_parameter("out", [TOK_PER_CORE, DIM], f32, isOutput=True)

    with tile.TileContext(nc) as tc:
        with (
            tc.tile_pool(name="const", bufs=1) as const_tp,
            tc.tile_pool(name="cp", bufs=8) as cp,
            tc.tile_pool(name="bbf", bufs=8) as bbf,
            tc.tile_pool(name="btp", bufs=8) as btp,
            tc.tile_pool(name="op", bufs=8) as op,
            tc.tile_pool(name="pst", bufs=2, space="PSUM") as pst,
            tc.tile_pool(name="psd", bufs=2, space="PSUM") as psd,
        ):
            from concourse.masks import make_identity

            identity = const_tp.tile([P, P], bf16)
            make_identity(nc, identity[:])

            ids_sb = const_tp.tile([P, N_TILES], mybir.dt.int32)
            nc.sync.dma_start(out=ids_sb[:], in_=ids_d[:])

            a_sb = const_tp.tile([RANK, DIM], f32)
            nc.sync.dma_start(out=a_sb[:], in_=a_d[:])
            a_bf = const_tp.tile([RANK, DIM], bf16)
            nc.vector.tensor_scalar_mul(a_bf[:], a_sb[:], SCALING)

            for g in range(N_TILES // GROUP):
                # One indirect DMA gathers GROUP*128 fused rows:
                # c_tile[p, k*ROW:(k+1)*ROW] = table[ids[p, g*GROUP+k], :]
                c_tile = cp.tile([P, GROUP * ROW], f32)
                nc.gpsimd.indirect_dma_start(
                    out=c_tile[:],
                    out_offset=None,
                    in_=t_d[:],
                    in_offset=bass.IndirectOffsetOnAxis(
                        ap=ids_sb[:, g * GROUP : (g + 1) * GROUP], axis=0
                    ),
                )
                for k in range(GROUP):
                    j = g * GROUP + k
                    w_ap = c_tile[:, k * ROW : k * ROW + DIM]
                    b_ap = c_tile[:, k * ROW + DIM : (k + 1) * ROW]

                    # cast + transpose on the (otherwise idle) Scalar engine
                    b_bf = bbf.tile([P, RANK], bf16)
                    nc.scalar.copy(out=b_bf[:], in_=b_ap)

                    # bT = b_bf.T : [RANK, P] so tokens land on PSUM partitions.
                    bT_ps = pst.tile([RANK, P], bf16)
                    nc.tensor.transpose(out=bT_ps[:], in_=b_bf[:], identity=identity[:])
                    bT = btp.tile([RANK, P], bf16)
                    nc.scalar.copy(out=bT[:], in_=bT_ps[:])

                    # delta = b @ (SCALING * lora_A) : [P, DIM], f32 accumulate
                    d_ps = psd.tile([P, DIM], f32)
                    out_tile = op.tile([P, DIM], f32)
                    for h in range(NSPLIT):
                        sl = slice(h * (DIM // NSPLIT), (h + 1) * (DIM // NSPLIT))
                        nc.tensor.matmul(
                            d_ps[:, sl], bT[:], a_bf[:, sl], start=True, stop=True
                        )
                        nc.vector.tensor_add(
                            out=out_tile[:, sl], in0=w_ap[:, sl], in1=d_ps[:, sl]
                        )
                    nc.sync.dma_start(out=out_d[j * P : (j + 1) * P, :], in_=out_tile[:])

    nc.compile()
    _cached_nc = nc
    return nc


def run(inputs, **spmd_kwargs):
    """Run on 8 cores; returns (full_output, BassKernelResults)."""
    ids = np.ascontiguousarray(np.asarray(inputs["input_ids"]).astype(np.int32)).reshape(-1)
    weight = np.asarray(inputs["weight"], dtype=np.float32)
    lora_a = np.ascontiguousarray(np.asarray(inputs["lora_A"], dtype=np.float32))
    lora_b = np.asarray(inputs["lora_B"], dtype=np.float32)
    assert ids.shape == (N_CORES * TOK_PER_CORE,)
    assert weight.shape == (VOCAB, DIM)
    assert lora_a.shape == (RANK, DIM)
    assert lora_b.shape == (VOCAB, RANK)
    table = np.ascontiguousarray(np.concatenate([weight, lora_b], axis=1))

    nc = _build_nc()
    in_maps = []
    for c in range(N_CORES):
        chunk = ids[c * TOK_PER_CORE : (c + 1) * TOK_PER_CORE]
        # ids_dev[p, j] = chunk[j * P + p] -> tile j gathers tokens j*P .. j*P+127
        ids_dev = np.ascontiguousarray(chunk.reshape(N_TILES, P).T)
        in_maps.append({"ids": ids_dev, "table": table, "lora_a": lora_a})
    res = run_bass_kernel_spmd(nc, in_maps, list(range(N_CORES)), **spmd_kwargs)
    out = np.stack([res.results[c]["out"] for c in range(N_CORES)], axis=0)
    return out.astype(np.float32, copy=False), res


def kernel(**inputs):
    out, _ = run(inputs)
    return out
